# revision 39
# baseline (speedup 1.0000x reference)
"""Multi-head attention (bz=2, slen=4096, d=768, 12 heads) on 8 trn2 NeuronCores.

Sharding: 8 cores = 2 (batch) x 2 (head halves of 6) x 2 (q halves of 2048).
Each core computes its q-slice of the partial output for its 6 heads; host sums
the two head-half partials per (batch, q-half) and adds b_o.

Key device-side structure (per core):
  - all matmul operands are bf16 (full-rate on PE, half the DMA/SBUF of fp32;
    measured end-to-end rel err ~5e-3 vs the 2e-2 gate); PSUM accumulates f32.
  - projections contract input features (768) on partitions; q/k inputs are fed
    pre-transposed [768, L] so qh^T/kh^T come out feature-major (QK operands),
    while vh is produced position-major (PV stationary operand) by using the
    transposed v as the stationary operand instead.
  - mask handling is exact and free: masked k-positions are gathered away on the
    host (softmax with -1e9 gives exactly 0 in fp32), and right-padding to a
    multiple of 128 is neutralized by a per-position "valid" column appended to
    v, which simultaneously produces the softmax row-sums during PV.
  - scores stay in S^T layout [k-pos partitions, q free]: softmax needs only one
    ACT pass (exp with fused 1/sqrt(64) scale); row-sums fall out of PV; the
    per-(head,q) normalization is applied to attn^T (64 rows) via
    reciprocal_approx_fast + gpsimd partition_broadcast + one DVE multiply.
  - QK packs head pairs into the PE array rows (K=64 x 2 via tile_position).
  - emission braiding: three lanes (projection chunks / attention rounds /
    o-projection) interleave fine-grained steps so the in-order PE and ACT
    queues never head-of-line block each other; o-projection no longer runs
    inside the attention lane (that starved the ACT engine between units).
"""

import os

import numpy as np

_CACHE = {}


def _build(KP):
    import concourse.mybir as mybir
    import concourse.tile as tile
    from concourse import bacc

    F32 = mybir.dt.float32
    BF16 = mybir.dt.bfloat16
    EXP = mybir.ActivationFunctionType.Exp

    F = 768          # model dim
    M = 384          # output features per core (6 heads x 64)
    QL = 2048        # q rows per core
    D = 64           # head dim
    H = 6            # heads per core
    KT = KP // 128   # k tiles
    QCW = 512        # chunk width (q chunks and projection column chunks)
    NQC = QL // QCW

    nc = bacc.Bacc("TRN2", target_bir_lowering=False, debug=False, num_devices=8)

    qT_d = nc.dram_tensor("qT", [F, QL], BF16, kind="ExternalInput").ap()
    kT_d = nc.dram_tensor("kT", [F, KP], BF16, kind="ExternalInput").ap()
    vT_d = nc.dram_tensor("vT", [F, KP], BF16, kind="ExternalInput").ap()
    wq_d = nc.dram_tensor("wq", [F, M], BF16, kind="ExternalInput").ap()
    wk_d = nc.dram_tensor("wk", [F, M], BF16, kind="ExternalInput").ap()
    wv_d = nc.dram_tensor("wv", [F, M], BF16, kind="ExternalInput").ap()
    wo_d = nc.dram_tensor("wo", [M, F], BF16, kind="ExternalInput").ap()
    valid_d = nc.dram_tensor("valid", [128, KT, 1], F32, kind="ExternalInput").ap()
    out_d = nc.dram_tensor("out", [QL, F], F32, kind="ExternalOutput").ap()

    with tile.TileContext(nc) as tc:
        with (
            tc.tile_pool(name="weights", bufs=1) as wp,
            tc.tile_pool(name="acts", bufs=1) as ap_,
            tc.tile_pool(name="ptp", bufs=20) as ptp,
            tc.tile_pool(name="small", bufs=4) as sp,
            tc.tile_pool(name="atp", bufs=2) as atp,
            tc.tile_pool(name="obp", bufs=2) as obp,
            tc.tile_pool(name="xc", bufs=3) as xcp,
            tc.tile_pool(name="xck", bufs=7) as xckp,
            tc.tile_pool(name="sps", bufs=2, space="PSUM") as sps,
            tc.tile_pool(name="acc", bufs=2, space="PSUM") as accp,
            tc.tile_pool(name="prj", bufs=2, space="PSUM") as prjp,
        ):
            # ---- weights: valid up front (tiny); wq/wk/wv/wo ride chunk
            # prefetches so the startup DMA bandwidth goes to the q0/k0/v0
            # chunks the first attention unit gates on
            valid_s = wp.tile([128, KT, 1], F32)
            nc.sync.dma_start(out=valid_s, in_=valid_d)
            wv_s = wp.tile([128, 6, M], BF16)
            wk_s = wp.tile([128, 6, M], BF16)
            wq_s = wp.tile([128, 6, M], BF16)
            wo_s = wp.tile([128, 3, F], BF16)

            def dma_w(dst, src):
                def go():
                    nc.sync.dma_start(
                        out=dst, in_=src.rearrange("(t p) m -> p t m", p=128))
                return go

            def dma_w_split(dst, src):
                # per-output-block pieces: the first projection matmul (m=0)
                # only waits for its own slice of the weights
                def go():
                    for m in range(3):
                        nc.sync.dma_start(
                            out=dst[:, :, m * 128:(m + 1) * 128],
                            in_=src[:, m * 128:(m + 1) * 128]
                            .rearrange("(t p) m -> p t m", p=128))
                return go

            dma_wq = dma_w_split(wq_s, wq_d)
            dma_wk = dma_w_split(wk_s, wk_d)
            dma_wv = dma_w(wv_s, wv_d)
            dma_wo = dma_w(wo_s, wo_d)

            # ---- resident projected activations ----
            qhT = ap_.tile([128, 3, QL], BF16)       # feature-major, head pair p
            khT = ap_.tile([128, 3, KP], BF16)
            vh = ap_.tile([128, KT, H, D + 1], BF16)  # position-major + valid col
            for h in range(H):
                nc.vector.tensor_copy(out=vh[:, :, h, D:D + 1], in_=valid_s)

            # readiness state: attention rounds spin-gate on these so their
            # emission order respects the progressive kh/vh/qh production
            # (kkt is per head-pair m so pair 0 can start before pairs 1/2
            # of the same k chunk have projected)
            st = {"vkt": 0, "kkt": [0, 0, 0], "qdone": set(), "fin": set()}

            class VUnit:
                kind = "v"

                def __init__(self, c0, cw, wdma=None):
                    self.c0 = c0
                    self.cw = cw
                    self.wdma = wdma

                def prefetch(self):
                    for w in self.wdma or ():
                        w()
                    self.xc = xcp.tile([128, 6, QCW], BF16, tag="xc",
                                       name=f"xcv{self.c0}")
                    nc.sync.dma_start(
                        out=self.xc[:, :, :self.cw],
                        in_=vT_d[:, self.c0:self.c0 + self.cw]
                        .rearrange("(t p) n -> p t n", p=128),
                    )

                def gen(self):
                    for t in range(self.cw // 128):
                        kt = self.c0 // 128 + t
                        ps = prjp.tile([128, M], F32, tag="prj")
                        for K in range(6):
                            nc.tensor.matmul(
                                ps,
                                lhsT=self.xc[:, K, t * 128:(t + 1) * 128],
                                rhs=wv_s[:, K, :],
                                start=(K == 0), stop=(K == 5),
                            )
                            if K % 2 == 1 and K < 5:
                                yield
                        nc.vector.tensor_copy(
                            out=vh[:, kt, :, 0:D],
                            in_=ps.rearrange("p (h d) -> p h d", h=H),
                        )
                        st["vkt"] += 1
                        yield

            class XUnit:
                def __init__(self, kind, xd, w_s, dst, c0, cw, wdma=None):
                    self.kind = kind
                    self.xd, self.w_s, self.dst, self.c0 = xd, w_s, dst, c0
                    self.cw = cw
                    self.wdma = wdma

                def prefetch(self):
                    for w in self.wdma or ():
                        w()
                    self.xc = xcp.tile([128, 6, QCW], BF16, tag="xc",
                                       name=f"xcx{id(self)}")
                    if self.kind == "q" and self.c0 == 0:
                        # piecewise so the very first projection matmul can
                        # start after ~0.5MB of DMA instead of the full chunk
                        for k2 in range(3):
                            nc.sync.dma_start(
                                out=self.xc[:, 2 * k2:2 * k2 + 2, :self.cw],
                                in_=self.xd[256 * k2:256 * (k2 + 1),
                                            self.c0:self.c0 + self.cw]
                                .rearrange("(t p) n -> p t n", p=128),
                            )
                    else:
                        nc.sync.dma_start(
                            out=self.xc[:, :, :self.cw],
                            in_=self.xd[:, self.c0:self.c0 + self.cw]
                            .rearrange("(t p) n -> p t n", p=128),
                        )

                def gen(self):
                    for m in range(3):
                        ps = prjp.tile([128, QCW], F32, tag="prj")
                        for K in range(6):
                            nc.tensor.matmul(
                                ps[:, :self.cw],
                                lhsT=self.w_s[:, K, m * 128:(m + 1) * 128],
                                rhs=self.xc[:, K, :self.cw],
                                start=(K == 0), stop=(K == 5),
                            )
                            if K % 2 == 1 and K < 5:
                                yield
                        nc.vector.tensor_copy(
                            out=self.dst[:, m, self.c0:self.c0 + self.cw],
                            in_=ps[:, :self.cw])
                        if self.kind == "q":
                            st["qdone"].add((self.c0 // QCW, m))
                        elif self.kind == "k":
                            st["kkt"][m] += self.cw // 128
                        yield

            # k-projection is split per (chunk, head-pair m): all m=0 pieces
            # emit first so attention unit (qc=0, p=0) unblocks after a short
            # k wave instead of waiting for the full k/v projection stream.
            # The chunk's activations are DMA'd once and stay resident for
            # the later m waves.
            xcks = {}

            class KMUnit:
                kind = "km"

                def __init__(self, ci, c0, cw, m, wdma=None):
                    self.ci, self.c0, self.cw, self.m = ci, c0, cw, m
                    self.wdma = wdma

                def prefetch(self):
                    for w in self.wdma or ():
                        w()
                    if self.m == 0:
                        xcks[self.ci] = xckp.tile(
                            [128, 6, QCW], BF16, tag="xck",
                            name=f"xck{self.ci}")
                        nc.sync.dma_start(
                            out=xcks[self.ci][:, :, :self.cw],
                            in_=kT_d[:, self.c0:self.c0 + self.cw]
                            .rearrange("(t p) n -> p t n", p=128),
                        )

                def gen(self):
                    m, xc = self.m, xcks[self.ci]
                    ps = prjp.tile([128, QCW], F32, tag="prj")
                    for K in range(6):
                        nc.tensor.matmul(
                            ps[:, :self.cw],
                            lhsT=wk_s[:, K, m * 128:(m + 1) * 128],
                            rhs=xc[:, K, :self.cw],
                            start=(K == 0), stop=(K == 5),
                        )
                        if K % 2 == 1 and K < 5:
                            yield
                    nc.vector.tensor_copy(
                        out=khT[:, m, self.c0:self.c0 + self.cw],
                        in_=ps[:, :self.cw])
                    st["kkt"][m] += self.cw // 128
                    yield

            at_tiles = {}

            # ---- streamed attention pipeline ----
            # Two coupled lanes over the same (qc, p, kt) stream: the QK lane
            # emits QK matmuls + exp as soon as kh tiles and a pt buffer are
            # available (keeping the ACT engine continuously fed, even while
            # the PE crunches v/k projections), and the PV lane trails behind
            # gated on vh production, draining pt buffers into the per-unit
            # PSUM accumulators. The pt pool depth bounds the lead. PV needs
            # no PSUM until it runs, so QK of later units overlaps PV of
            # earlier ones without extra PSUM banks.
            units = [(qc, p) for qc in range(NQC) for p in range(3)]
            ptq = []
            stc = {"qk": 0, "pv": 0}
            PT_B = 20
            TOT = len(units) * KT

            def qk_stream():
                for qc, p in units:
                    q0 = qc * QCW
                    while (qc, p) not in st["qdone"]:
                        yield
                    for kt in range(KT):
                        while kt >= st["kkt"][p]:
                            yield
                        lag = min(PT_B - 1, max(1, TOT - stc["qk"]))
                        while stc["qk"] - stc["pv"] >= lag:
                            yield
                        ps = sps.tile([128, 2, QCW], F32, tag="ps")
                        for j in range(2):
                            nc.tensor.matmul(
                                ps[:, j, :],
                                lhsT=khT[j * 64:(j + 1) * 64, p,
                                         kt * 128:(kt + 1) * 128],
                                rhs=qhT[j * 64:(j + 1) * 64, p, q0:q0 + QCW],
                                start=True, stop=True,
                            )
                        pt = ptp.tile([128, 2, QCW], BF16, tag="pt")
                        nc.scalar.activation(pt, ps, EXP, scale=0.125)
                        ptq.append(pt)
                        stc["qk"] += 1
                        yield

            def finalize(qc, p, pacc):
                if qc not in at_tiles:
                    at_tiles[qc] = atp.tile([128, 3, QCW], BF16, tag="at",
                                            name=f"at{qc}")
                at = at_tiles[qc]
                # copy the accumulators out of PSUM right away (frees the
                # pacc bank for the next unit ~1us after the last PV instead
                # of after the whole normalize chain), then normalize from
                # SBUF (the custom-DVE reciprocal misreads PSUM on hw anyway)
                cps, sms = [], []
                for j in range(2):
                    # partition-0 staging for the sums first (the custom-DVE
                    # reciprocal misreads inputs at a nonzero base partition);
                    # emitting these before the big copies starts the recip
                    # chain earlier
                    sm = sp.tile([1, QCW], F32, tag="sm",
                                 name=f"sm{qc}_{p}_{j}")
                    nc.vector.tensor_copy(out=sm, in_=pacc[j][D:D + 1, :])
                    sms.append(sm)
                for j in range(2):
                    cp = sp.tile([D + 1, QCW], F32, tag="cp",
                                 name=f"cp{qc}_{p}_{j}")
                    nc.vector.tensor_copy(out=cp, in_=pacc[j])
                    cps.append(cp)
                for j in range(2):
                    rr = sp.tile([1, QCW], F32, tag="rr")
                    if os.environ.get("KERNEL_EXACT_RECIP"):
                        nc.vector.reciprocal(rr, sms[j])
                    else:
                        nc.vector.reciprocal_approx_fast(out=rr, in_=sms[j])
                    bc = sp.tile([64, QCW], F32, tag="bc")
                    nc.gpsimd.partition_broadcast(bc, rr)
                    nc.vector.tensor_mul(
                        at[j * 64:(j + 1) * 64, p, :],
                        cps[j][0:D, :], bc)
                st["fin"].add((qc, p))
                remaining[qc] -= 1
                if remaining[qc] == 0:
                    ready_oproj.append(OprojUnit(qc))

            def pv_stream():
                for qc, p in units:
                    pacc = None
                    for kt in range(KT):
                        while stc["qk"] <= stc["pv"]:
                            yield
                        while kt >= st["vkt"]:
                            yield
                        if pacc is None:
                            pacc = (accp.tile([D + 1, QCW], F32, tag="acc",
                                              name=f"pa{qc}_{p}"),
                                    accp.tile([D + 1, QCW], F32, tag="acc",
                                              name=f"pb{qc}_{p}"))
                        pt = ptq.pop(0)
                        for j in range(2):
                            nc.tensor.matmul(
                                pacc[j],
                                lhsT=vh[:, kt, 2 * p + j, :],
                                rhs=pt[:, j, :],
                                start=(kt == 0), stop=(kt == KT - 1),
                            )
                        stc["pv"] += 1
                        if kt == KT - 1:
                            finalize(qc, p, pacc)
                            # give the QK lane a head start before the next
                            # unit's first PV (which waits on the pacc copy)
                            yield
                            yield
                        yield

            class OprojUnit:
                kind = "oproj"

                def __init__(self, qc):
                    self.qc = qc

                def gen(self):
                    qc, q0 = self.qc, self.qc * QCW
                    at = at_tiles[qc]
                    for m2 in range(QCW // 128):
                        ob = obp.tile([128, F], F32, tag="ob")
                        for nch in range(2):
                            po = prjp.tile([128, 384], F32, tag="prj")
                            for t in range(3):
                                while (qc, t) not in st["fin"]:
                                    yield
                                nc.tensor.matmul(
                                    po,
                                    lhsT=at[:, t, m2 * 128:(m2 + 1) * 128],
                                    rhs=wo_s[:, t, nch * 384:(nch + 1) * 384],
                                    start=(t == 0), stop=(t == 2),
                                )
                            nc.vector.tensor_copy(
                                out=ob[:, nch * 384:(nch + 1) * 384], in_=po)
                            nc.sync.dma_start(
                                out=out_d[q0 + m2 * 128:q0 + (m2 + 1) * 128,
                                          nch * 384:(nch + 1) * 384],
                                in_=ob[:, nch * 384:(nch + 1) * 384],
                            )
                            yield

            # ---- queues ----
            # head chunks are narrow so the first attention rounds (which
            # gate on k/v tile production) can start as early as possible
            def chunk_spans(L):
                spans, c = [], 0
                for w in (128, 128, 256):
                    if c < L:
                        w2 = min(w, L - c)
                        spans.append((c, w2))
                        c += w2
                while c < L:
                    w2 = min(QCW, L - c)
                    spans.append((c, w2))
                    c += w2
                return spans

            spans = chunk_spans(KP)
            vq = [VUnit(c0, cw, wdma=[dma_wv] if i == 0 else None)
                  for i, (c0, cw) in enumerate(spans)]
            kq = {(i, m): KMUnit(i, c0, cw, m,
                                 wdma=[dma_wk] if (i, m) == (0, 0) else None)
                  for i, (c0, cw) in enumerate(spans) for m in range(3)}
            kq[(min(3, len(spans) - 1), 0)].wdma = \
                (kq[(min(3, len(spans) - 1), 0)].wdma or []) + [dma_wo]
            qunits = [XUnit("q", qT_d, wq_s, qhT, c * QCW, QCW,
                            wdma=[dma_wq] if c == 0 else None)
                      for c in range(NQC)]
            # q0 first (gates attention start), then the m=0 k wave braided
            # with the v chunks, then the m=1 / m=2 k waves and later q
            # chunks (consumed during earlier attention units)
            chunk_q = [qunits[0]]
            for i in range(max(len(vq), len(spans))):
                if i < len(spans):
                    chunk_q.append(kq[(i, 0)])
                if i < len(vq):
                    chunk_q.append(vq[i])
            chunk_q.extend(kq[(i, 1)] for i in range(len(spans)))
            for n, qu in enumerate(qunits[1:], 1):
                chunk_q.append(qu)
                if n == 1:
                    chunk_q.extend(kq[(i, 2)] for i in range(len(spans)))

            def oproj_lane():
                done = 0
                while done < NQC:
                    if ready_oproj:
                        u = ready_oproj.pop(0)
                        # let the finalize DVE chain drain before queueing
                        # oproj matmuls on the PE (head-of-line blocking);
                        # no delay for the last unit (pure tail)
                        if done < NQC - 1:
                            for _ in range(3):
                                yield
                        yield from u.gen()
                        done += 1
                    else:
                        yield

            remaining = {qc: 3 for qc in range(NQC)}
            ready_oproj = []
            idx = {"c": 0, "cpf": 0}

            def ensure_pf(upto):
                while idx["cpf"] < min(upto, len(chunk_q)):
                    u = chunk_q[idx["cpf"]]
                    idx["cpf"] += 1
                    if hasattr(u, "prefetch"):
                        u.prefetch()

            def chunk_lane():
                while idx["c"] < len(chunk_q):
                    u = chunk_q[idx["c"]]
                    idx["c"] += 1
                    ensure_pf(idx["c"] + 2)
                    yield from u.gen()

            ensure_pf(2)
            # braid order per round: QK/exp first (feeds ACT), then PV,
            # then projection chunks, then o-projection
            active = [qk_stream(), pv_stream(), chunk_lane(), oproj_lane()]
            while active:
                nxt = []
                for g in active:
                    try:
                        next(g)
                        nxt.append(g)
                    except StopIteration:
                        pass
                active = nxt

    nc.compile()
    return nc


last_results = None


def _ensure_ntff_hook():
    """Install the axon NTFF profile hook if the image's antenv lacks it.

    trn_agent_boot intends to register this hook (see trn_boot.py step 6); on
    images whose antenv has no axon_hooks module it degrades. Recreate the
    module so trace=True works; silently no-op if anything is unavailable.
    """
    import sys
    import types
    try:
        import antenv.axon_hooks  # noqa: F401
        return
    except ImportError:
        pass
    try:
        import antenv
        from trn_agent_boot.trn_boot import _ntff_profile_via_ctypes
        hook = _ntff_profile_via_ctypes("/opt/axon/libaxon_pjrt.so")
        mod = types.ModuleType("antenv.axon_hooks")
        mod.get_axon_ntff_profile_hook = lambda: hook
        mod.set_axon_ntff_profile_hook = lambda h: None
        sys.modules["antenv.axon_hooks"] = mod
        antenv.axon_hooks = mod
    except Exception:
        pass


def kernel(v, k, q, mask, w_v, w_k, w_q, w_o, b_o):
    global last_results
    import ml_dtypes
    from concourse import bass_utils

    BF = ml_dtypes.bfloat16

    v = np.asarray(v, dtype=np.float32)
    k = np.asarray(k, dtype=np.float32)
    q = np.asarray(q, dtype=np.float32)
    mask = np.asarray(mask)
    w_v = np.asarray(w_v, dtype=np.float32)
    w_k = np.asarray(w_k, dtype=np.float32)
    w_q = np.asarray(w_q, dtype=np.float32)
    w_o = np.asarray(w_o, dtype=np.float32)
    b_o = np.asarray(b_o, dtype=np.float32)

    BZ, SL, F = q.shape
    QL = SL // 2

    kept = [np.flatnonzero(mask[b, 0, 0] == 0) for b in range(BZ)]
    klens = [len(x) for x in kept]
    KP = max(128, -(-max(klens) // 128) * 128)
    KT = KP // 128

    if KP not in _CACHE:
        _CACHE[KP] = _build(KP)
    nc = _CACHE[KP]

    # per-batch gathered/padded transposed k/v and validity tables
    kTs, vTs, valids = [], [], []
    for b in range(BZ):
        kt = np.zeros((F, KP), BF)
        vt = np.zeros((F, KP), BF)
        kt[:, :klens[b]] = k[b, kept[b]].T.astype(BF)
        vt[:, :klens[b]] = v[b, kept[b]].T.astype(BF)
        kTs.append(kt)
        vTs.append(vt)
        val = np.zeros(KP, np.float32)
        val[:klens[b]] = 1.0
        valids.append(np.ascontiguousarray(val.reshape(KT, 128).T)[:, :, None])

    in_maps = []
    for c in range(8):
        b, hg, qg = c // 4, (c // 2) % 2, c % 2
        mc = slice(384 * hg, 384 * (hg + 1))
        qr = slice(QL * qg, QL * (qg + 1))
        in_maps.append({
            "qT": np.ascontiguousarray(q[b, qr].T.astype(BF)),
            "kT": kTs[b],
            "vT": vTs[b],
            "wq": np.ascontiguousarray(w_q[:, mc].astype(BF)),
            "wk": np.ascontiguousarray(w_k[:, mc].astype(BF)),
            "wv": np.ascontiguousarray(w_v[:, mc].astype(BF)),
            "wo": np.ascontiguousarray(w_o[mc, :].astype(BF)),
            "valid": valids[b],
        })

    trace = bool(os.environ.get("KERNEL_TRACE")) or bool(os.environ.get("BASS_TRACE"))
    if trace:
        _ensure_ntff_hook()
    tcores = os.environ.get("KERNEL_TRACE_CORES", "0,1,2,3,4,5,6,7")
    tcores = [int(x) for x in tcores.split(",")]
    try:
        res = bass_utils.run_bass_kernel_spmd(
            nc, in_maps, core_ids=list(range(8)),
            trace=trace,
            trace_cores=tcores if trace else None,
        )
    except Exception:
        if not trace:
            raise
        os.environ["BASS_NEVER_TRACE"] = "1"
        try:
            res = bass_utils.run_bass_kernel_spmd(
                nc, in_maps, core_ids=list(range(8)),
            )
        finally:
            del os.environ["BASS_NEVER_TRACE"]
    last_results = res

    out = np.empty((BZ, SL, F), np.float32)
    for b in range(BZ):
        for qg in range(2):
            c0 = b * 4 + qg          # hg = 0
            c1 = b * 4 + 2 + qg      # hg = 1
            out[b, QL * qg:QL * (qg + 1)] = (
                res.results[c0]["out"] + res.results[c1]["out"] + b_o
            )
    return out


# revision 40
# speedup vs baseline: 1.0121x; 1.0121x over previous
"""Multi-head attention (bz=2, slen=4096, d=768, 12 heads) on 8 trn2 NeuronCores.

Sharding: 8 cores = 2 (batch) x 2 (head halves of 6) x 2 (q halves of 2048).
Each core computes its q-slice of the partial output for its 6 heads; host sums
the two head-half partials per (batch, q-half) and adds b_o.

Key device-side structure (per core):
  - all matmul operands are bf16 (full-rate on PE, half the DMA/SBUF of fp32;
    measured end-to-end rel err ~5e-3 vs the 2e-2 gate); PSUM accumulates f32.
  - projections contract input features (768) on partitions; q/k inputs are fed
    pre-transposed [768, L] so qh^T/kh^T come out feature-major (QK operands),
    while vh is produced position-major (PV stationary operand) by using the
    transposed v as the stationary operand instead.
  - mask handling is exact and free: masked k-positions are gathered away on the
    host (softmax with -1e9 gives exactly 0 in fp32), and right-padding to a
    multiple of 128 is neutralized by a per-position "valid" column appended to
    v, which simultaneously produces the softmax row-sums during PV.
  - scores stay in S^T layout [k-pos partitions, q free]: softmax needs only one
    ACT pass (exp with fused 1/sqrt(64) scale); row-sums fall out of PV; the
    per-(head,q) normalization is applied to attn^T (64 rows) via
    reciprocal_approx_fast + gpsimd partition_broadcast + one DVE multiply.
  - QK packs head pairs into the PE array rows (K=64 x 2 via tile_position).
  - emission braiding: three lanes (projection chunks / attention rounds /
    o-projection) interleave fine-grained steps so the in-order PE and ACT
    queues never head-of-line block each other; o-projection no longer runs
    inside the attention lane (that starved the ACT engine between units).
"""

import os

import numpy as np

_CACHE = {}


def _build(KP):
    import concourse.mybir as mybir
    import concourse.tile as tile
    from concourse import bacc

    F32 = mybir.dt.float32
    BF16 = mybir.dt.bfloat16
    EXP = mybir.ActivationFunctionType.Exp

    F = 768          # model dim
    M = 384          # output features per core (6 heads x 64)
    QL = 2048        # q rows per core
    D = 64           # head dim
    H = 6            # heads per core
    KT = KP // 128   # k tiles
    QCW = 512        # chunk width (q chunks and projection column chunks)
    NQC = QL // QCW

    nc = bacc.Bacc("TRN2", target_bir_lowering=False, debug=False, num_devices=8)

    qT_d = nc.dram_tensor("qT", [F, QL], BF16, kind="ExternalInput").ap()
    kT_d = nc.dram_tensor("kT", [F, KP], BF16, kind="ExternalInput").ap()
    vT_d = nc.dram_tensor("vT", [F, KP], BF16, kind="ExternalInput").ap()
    wq_d = nc.dram_tensor("wq", [F, M], BF16, kind="ExternalInput").ap()
    wk_d = nc.dram_tensor("wk", [F, M], BF16, kind="ExternalInput").ap()
    wv_d = nc.dram_tensor("wv", [F, M], BF16, kind="ExternalInput").ap()
    wo_d = nc.dram_tensor("wo", [M, F], BF16, kind="ExternalInput").ap()
    valid_d = nc.dram_tensor("valid", [128, KT, 1], F32, kind="ExternalInput").ap()
    out_d = nc.dram_tensor("out", [QL, F], F32, kind="ExternalOutput").ap()

    with tile.TileContext(nc) as tc:
        with (
            tc.tile_pool(name="weights", bufs=1) as wp,
            tc.tile_pool(name="acts", bufs=1) as ap_,
            tc.tile_pool(name="ptp", bufs=20) as ptp,
            tc.tile_pool(name="small", bufs=4) as sp,
            tc.tile_pool(name="atp", bufs=2) as atp,
            tc.tile_pool(name="obp", bufs=2) as obp,
            tc.tile_pool(name="xc", bufs=3) as xcp,
            tc.tile_pool(name="xck", bufs=7) as xckp,
            tc.tile_pool(name="sps", bufs=2, space="PSUM") as sps,
            tc.tile_pool(name="acc", bufs=2, space="PSUM") as accp,
            tc.tile_pool(name="prj", bufs=2, space="PSUM") as prjp,
        ):
            # ---- weights: valid up front (tiny); wq/wk/wv/wo ride chunk
            # prefetches so the startup DMA bandwidth goes to the q0/k0/v0
            # chunks the first attention unit gates on
            valid_s = wp.tile([128, KT, 1], F32)
            nc.sync.dma_start(out=valid_s, in_=valid_d)
            wv_s = wp.tile([128, 6, M], BF16)
            wk_s = wp.tile([128, 6, M], BF16)
            wq_s = wp.tile([128, 6, M], BF16)
            wo_s = wp.tile([128, 3, F], BF16)

            def dma_w(dst, src):
                def go():
                    nc.sync.dma_start(
                        out=dst, in_=src.rearrange("(t p) m -> p t m", p=128))
                return go

            def dma_w_split(dst, src):
                # per-output-block pieces: the first projection matmul (m=0)
                # only waits for its own slice of the weights
                def go():
                    for m in range(3):
                        nc.sync.dma_start(
                            out=dst[:, :, m * 128:(m + 1) * 128],
                            in_=src[:, m * 128:(m + 1) * 128]
                            .rearrange("(t p) m -> p t m", p=128))
                return go

            dma_wq = dma_w_split(wq_s, wq_d)
            dma_wk = dma_w_split(wk_s, wk_d)
            dma_wv = dma_w(wv_s, wv_d)
            dma_wo = dma_w(wo_s, wo_d)

            # ---- resident projected activations ----
            qhT = ap_.tile([128, 3, QL], BF16)       # feature-major, head pair p
            khT = ap_.tile([128, 3, KP], BF16)
            vh = ap_.tile([128, KT, H, D + 1], BF16)  # position-major + valid col
            for h in range(H):
                nc.vector.tensor_copy(out=vh[:, :, h, D:D + 1], in_=valid_s)

            # readiness state: attention rounds spin-gate on these so their
            # emission order respects the progressive kh/vh/qh production
            # (kkt is per head-pair m so pair 0 can start before pairs 1/2
            # of the same k chunk have projected)
            st = {"vkt": 0, "kkt": [0, 0, 0], "qdone": set(), "fin": set()}

            class VUnit:
                kind = "v"

                def __init__(self, c0, cw, wdma=None):
                    self.c0 = c0
                    self.cw = cw
                    self.wdma = wdma

                def prefetch(self):
                    for w in self.wdma or ():
                        w()
                    self.xc = xcp.tile([128, 6, QCW], BF16, tag="xc",
                                       name=f"xcv{self.c0}")
                    nc.sync.dma_start(
                        out=self.xc[:, :, :self.cw],
                        in_=vT_d[:, self.c0:self.c0 + self.cw]
                        .rearrange("(t p) n -> p t n", p=128),
                    )

                def gen(self):
                    for t in range(self.cw // 128):
                        kt = self.c0 // 128 + t
                        ps = prjp.tile([128, M], F32, tag="prj")
                        for K in range(6):
                            nc.tensor.matmul(
                                ps,
                                lhsT=self.xc[:, K, t * 128:(t + 1) * 128],
                                rhs=wv_s[:, K, :],
                                start=(K == 0), stop=(K == 5),
                            )
                            if K % 2 == 1 and K < 5:
                                yield
                        nc.vector.tensor_copy(
                            out=vh[:, kt, :, 0:D],
                            in_=ps.rearrange("p (h d) -> p h d", h=H),
                        )
                        st["vkt"] += 1
                        yield

            class XUnit:
                def __init__(self, kind, xd, w_s, dst, c0, cw, wdma=None):
                    self.kind = kind
                    self.xd, self.w_s, self.dst, self.c0 = xd, w_s, dst, c0
                    self.cw = cw
                    self.wdma = wdma

                def prefetch(self):
                    for w in self.wdma or ():
                        w()
                    self.xc = xcp.tile([128, 6, QCW], BF16, tag="xc",
                                       name=f"xcx{id(self)}")
                    if self.kind == "q" and self.c0 == 0:
                        # piecewise so the very first projection matmul can
                        # start after ~0.5MB of DMA instead of the full chunk
                        for k2 in range(3):
                            nc.sync.dma_start(
                                out=self.xc[:, 2 * k2:2 * k2 + 2, :self.cw],
                                in_=self.xd[256 * k2:256 * (k2 + 1),
                                            self.c0:self.c0 + self.cw]
                                .rearrange("(t p) n -> p t n", p=128),
                            )
                    else:
                        nc.sync.dma_start(
                            out=self.xc[:, :, :self.cw],
                            in_=self.xd[:, self.c0:self.c0 + self.cw]
                            .rearrange("(t p) n -> p t n", p=128),
                        )

                def gen(self):
                    for m in range(3):
                        ps = prjp.tile([128, QCW], F32, tag="prj")
                        for K in range(6):
                            nc.tensor.matmul(
                                ps[:, :self.cw],
                                lhsT=self.w_s[:, K, m * 128:(m + 1) * 128],
                                rhs=self.xc[:, K, :self.cw],
                                start=(K == 0), stop=(K == 5),
                            )
                            if K % 2 == 1 and K < 5:
                                yield
                        nc.vector.tensor_copy(
                            out=self.dst[:, m, self.c0:self.c0 + self.cw],
                            in_=ps[:, :self.cw])
                        if self.kind == "q":
                            st["qdone"].add((self.c0 // QCW, m))
                        elif self.kind == "k":
                            st["kkt"][m] += self.cw // 128
                        yield

            # k-projection is split per (chunk, head-pair m): all m=0 pieces
            # emit first so attention unit (qc=0, p=0) unblocks after a short
            # k wave instead of waiting for the full k/v projection stream.
            # The chunk's activations are DMA'd once and stay resident for
            # the later m waves.
            xcks = {}

            class KMUnit:
                kind = "km"

                def __init__(self, ci, c0, cw, m, wdma=None):
                    self.ci, self.c0, self.cw, self.m = ci, c0, cw, m
                    self.wdma = wdma

                def prefetch(self):
                    for w in self.wdma or ():
                        w()
                    if self.m == 0:
                        xcks[self.ci] = xckp.tile(
                            [128, 6, QCW], BF16, tag="xck",
                            name=f"xck{self.ci}")
                        nc.sync.dma_start(
                            out=xcks[self.ci][:, :, :self.cw],
                            in_=kT_d[:, self.c0:self.c0 + self.cw]
                            .rearrange("(t p) n -> p t n", p=128),
                        )

                def gen(self):
                    m, xc = self.m, xcks[self.ci]
                    ps = prjp.tile([128, QCW], F32, tag="prj")
                    for K in range(6):
                        nc.tensor.matmul(
                            ps[:, :self.cw],
                            lhsT=wk_s[:, K, m * 128:(m + 1) * 128],
                            rhs=xc[:, K, :self.cw],
                            start=(K == 0), stop=(K == 5),
                        )
                        if K % 2 == 1 and K < 5:
                            yield
                    nc.vector.tensor_copy(
                        out=khT[:, m, self.c0:self.c0 + self.cw],
                        in_=ps[:, :self.cw])
                    st["kkt"][m] += self.cw // 128
                    yield

            at_tiles = {}

            # ---- streamed attention pipeline ----
            # Two coupled lanes over the same (qc, p, kt) stream: the QK lane
            # emits QK matmuls + exp as soon as kh tiles and a pt buffer are
            # available (keeping the ACT engine continuously fed, even while
            # the PE crunches v/k projections), and the PV lane trails behind
            # gated on vh production, draining pt buffers into the per-unit
            # PSUM accumulators. The pt pool depth bounds the lead. PV needs
            # no PSUM until it runs, so QK of later units overlaps PV of
            # earlier ones without extra PSUM banks.
            units = [(qc, p) for qc in range(NQC) for p in range(3)]
            ptq = []
            stc = {"qk": 0, "pv": 0}
            PT_B = 20
            TOT = len(units) * KT

            def qk_stream():
                for qc, p in units:
                    q0 = qc * QCW
                    while (qc, p) not in st["qdone"]:
                        yield
                    for kt in range(KT):
                        while kt >= st["kkt"][p]:
                            yield
                        lag = min(PT_B - 1, max(1, TOT - stc["qk"]))
                        while stc["qk"] - stc["pv"] >= lag:
                            yield
                        ps = sps.tile([128, 2, QCW], F32, tag="ps")
                        for j in range(2):
                            nc.tensor.matmul(
                                ps[:, j, :],
                                lhsT=khT[j * 64:(j + 1) * 64, p,
                                         kt * 128:(kt + 1) * 128],
                                rhs=qhT[j * 64:(j + 1) * 64, p, q0:q0 + QCW],
                                start=True, stop=True,
                            )
                        pt = ptp.tile([128, 2, QCW], BF16, tag="pt")
                        nc.scalar.activation(pt, ps, EXP, scale=0.125)
                        ptq.append(pt)
                        stc["qk"] += 1
                        yield

            def finalize(qc, p, pacc):
                if qc not in at_tiles:
                    at_tiles[qc] = atp.tile([128, 3, QCW], BF16, tag="at",
                                            name=f"at{qc}")
                at = at_tiles[qc]
                # copy the accumulators out of PSUM right away (frees the
                # pacc bank for the next unit ~1us after the last PV instead
                # of after the whole normalize chain), then normalize from
                # SBUF (the custom-DVE reciprocal misreads PSUM on hw anyway)
                cps, sms = [], []
                for j in range(2):
                    # partition-0 staging for the sums first (the custom-DVE
                    # reciprocal misreads inputs at a nonzero base partition);
                    # emitting these before the big copies starts the recip
                    # chain earlier
                    sm = sp.tile([1, QCW], F32, tag="sm",
                                 name=f"sm{qc}_{p}_{j}")
                    nc.vector.tensor_copy(out=sm, in_=pacc[j][D:D + 1, :])
                    sms.append(sm)
                for j in range(2):
                    cp = sp.tile([D + 1, QCW], F32, tag="cp",
                                 name=f"cp{qc}_{p}_{j}")
                    nc.vector.tensor_copy(out=cp, in_=pacc[j])
                    cps.append(cp)
                for j in range(2):
                    rr = sp.tile([1, QCW], F32, tag="rr")
                    if os.environ.get("KERNEL_EXACT_RECIP"):
                        nc.vector.reciprocal(rr, sms[j])
                    else:
                        nc.vector.reciprocal_approx_fast(out=rr, in_=sms[j])
                    bc = sp.tile([64, QCW], F32, tag="bc")
                    nc.gpsimd.partition_broadcast(bc, rr)
                    nc.vector.tensor_mul(
                        at[j * 64:(j + 1) * 64, p, :],
                        cps[j][0:D, :], bc)
                st["fin"].add((qc, p))
                remaining[qc] -= 1
                if remaining[qc] == 0:
                    ready_oproj.append(OprojUnit(qc))

            def pv_stream():
                for qc, p in units:
                    pacc = None
                    for kt in range(KT):
                        while stc["qk"] <= stc["pv"]:
                            yield
                        while kt >= st["vkt"]:
                            yield
                        if pacc is None:
                            pacc = (accp.tile([D + 1, QCW], F32, tag="acc",
                                              name=f"pa{qc}_{p}"),
                                    accp.tile([D + 1, QCW], F32, tag="acc",
                                              name=f"pb{qc}_{p}"))
                        pt = ptq.pop(0)
                        for j in range(2):
                            nc.tensor.matmul(
                                pacc[j],
                                lhsT=vh[:, kt, 2 * p + j, :],
                                rhs=pt[:, j, :],
                                start=(kt == 0), stop=(kt == KT - 1),
                            )
                        stc["pv"] += 1
                        if kt == KT - 1:
                            finalize(qc, p, pacc)
                            # give the QK lane a head start before the next
                            # unit's first PV (which waits on the pacc copy)
                            yield
                            yield
                        yield

            class OprojUnit:
                kind = "oproj"

                def __init__(self, qc):
                    self.qc = qc

                def gen(self):
                    qc, q0 = self.qc, self.qc * QCW
                    at = at_tiles[qc]
                    for m2 in range(QCW // 128):
                        ob = obp.tile([128, F], F32, tag="ob")
                        for nch in range(2):
                            po = prjp.tile([128, 384], F32, tag="prj")
                            for t in range(3):
                                while (qc, t) not in st["fin"]:
                                    yield
                                nc.tensor.matmul(
                                    po,
                                    lhsT=at[:, t, m2 * 128:(m2 + 1) * 128],
                                    rhs=wo_s[:, t, nch * 384:(nch + 1) * 384],
                                    start=(t == 0), stop=(t == 2),
                                )
                            nc.vector.tensor_copy(
                                out=ob[:, nch * 384:(nch + 1) * 384], in_=po)
                            nc.sync.dma_start(
                                out=out_d[q0 + m2 * 128:q0 + (m2 + 1) * 128,
                                          nch * 384:(nch + 1) * 384],
                                in_=ob[:, nch * 384:(nch + 1) * 384],
                            )
                            yield

            # ---- queues ----
            # head chunks are narrow so the first attention rounds (which
            # gate on k/v tile production) can start as early as possible
            def chunk_spans(L):
                spans, c = [], 0
                for w in (128, 128, 256):
                    if c < L:
                        w2 = min(w, L - c)
                        spans.append((c, w2))
                        c += w2
                while c < L:
                    w2 = min(QCW, L - c)
                    spans.append((c, w2))
                    c += w2
                return spans

            spans = chunk_spans(KP)
            vq = [VUnit(c0, cw, wdma=[dma_wv] if i == 0 else None)
                  for i, (c0, cw) in enumerate(spans)]
            kq = {(i, m): KMUnit(i, c0, cw, m,
                                 wdma=[dma_wk] if (i, m) == (0, 0) else None)
                  for i, (c0, cw) in enumerate(spans) for m in range(3)}
            kq[(min(3, len(spans) - 1), 0)].wdma = \
                (kq[(min(3, len(spans) - 1), 0)].wdma or []) + [dma_wo]
            qunits = [XUnit("q", qT_d, wq_s, qhT, c * QCW, QCW,
                            wdma=[dma_wq] if c == 0 else None)
                      for c in range(NQC)]
            # q0 first (gates attention start), then the m=0 k wave braided
            # with the v chunks, then the m=1 / m=2 k waves and later q
            # chunks (consumed during earlier attention units)
            chunk_q = [qunits[0]]
            for i in range(max(len(vq), len(spans))):
                if i < len(spans):
                    chunk_q.append(kq[(i, 0)])
                if i < len(vq):
                    chunk_q.append(vq[i])
            chunk_q.append(qunits[1])
            chunk_q.extend(kq[(i, 1)] for i in range(len(spans)))
            chunk_q.append(qunits[2])
            chunk_q.extend(kq[(i, 2)] for i in range(len(spans)))
            chunk_q.extend(qunits[3:])

            def oproj_lane():
                done = 0
                while done < NQC:
                    if ready_oproj:
                        u = ready_oproj.pop(0)
                        # let the finalize DVE chain drain before queueing
                        # oproj matmuls on the PE (head-of-line blocking);
                        # no delay for the last unit (pure tail)
                        if done < NQC - 1:
                            for _ in range(3):
                                yield
                        yield from u.gen()
                        done += 1
                    else:
                        yield

            remaining = {qc: 3 for qc in range(NQC)}
            ready_oproj = []
            idx = {"c": 0, "cpf": 0}

            def ensure_pf(upto):
                while idx["cpf"] < min(upto, len(chunk_q)):
                    u = chunk_q[idx["cpf"]]
                    idx["cpf"] += 1
                    if hasattr(u, "prefetch"):
                        u.prefetch()

            def chunk_lane():
                while idx["c"] < len(chunk_q):
                    u = chunk_q[idx["c"]]
                    idx["c"] += 1
                    ensure_pf(idx["c"] + 2)
                    yield from u.gen()

            ensure_pf(2)
            # braid order per round: QK/exp first (feeds ACT), then PV,
            # then projection chunks, then o-projection
            active = [qk_stream(), pv_stream(), chunk_lane(), oproj_lane()]
            while active:
                nxt = []
                for g in active:
                    try:
                        next(g)
                        nxt.append(g)
                    except StopIteration:
                        pass
                active = nxt

    nc.compile()
    return nc


last_results = None


def _ensure_ntff_hook():
    """Install the axon NTFF profile hook if the image's antenv lacks it.

    trn_agent_boot intends to register this hook (see trn_boot.py step 6); on
    images whose antenv has no axon_hooks module it degrades. Recreate the
    module so trace=True works; silently no-op if anything is unavailable.
    """
    import sys
    import types
    try:
        import antenv.axon_hooks  # noqa: F401
        return
    except ImportError:
        pass
    try:
        import antenv
        from trn_agent_boot.trn_boot import _ntff_profile_via_ctypes
        hook = _ntff_profile_via_ctypes("/opt/axon/libaxon_pjrt.so")
        mod = types.ModuleType("antenv.axon_hooks")
        mod.get_axon_ntff_profile_hook = lambda: hook
        mod.set_axon_ntff_profile_hook = lambda h: None
        sys.modules["antenv.axon_hooks"] = mod
        antenv.axon_hooks = mod
    except Exception:
        pass


def kernel(v, k, q, mask, w_v, w_k, w_q, w_o, b_o):
    global last_results
    import ml_dtypes
    from concourse import bass_utils

    BF = ml_dtypes.bfloat16

    v = np.asarray(v, dtype=np.float32)
    k = np.asarray(k, dtype=np.float32)
    q = np.asarray(q, dtype=np.float32)
    mask = np.asarray(mask)
    w_v = np.asarray(w_v, dtype=np.float32)
    w_k = np.asarray(w_k, dtype=np.float32)
    w_q = np.asarray(w_q, dtype=np.float32)
    w_o = np.asarray(w_o, dtype=np.float32)
    b_o = np.asarray(b_o, dtype=np.float32)

    BZ, SL, F = q.shape
    QL = SL // 2

    kept = [np.flatnonzero(mask[b, 0, 0] == 0) for b in range(BZ)]
    klens = [len(x) for x in kept]
    KP = max(128, -(-max(klens) // 128) * 128)
    KT = KP // 128

    if KP not in _CACHE:
        _CACHE[KP] = _build(KP)
    nc = _CACHE[KP]

    # per-batch gathered/padded transposed k/v and validity tables
    kTs, vTs, valids = [], [], []
    for b in range(BZ):
        kt = np.zeros((F, KP), BF)
        vt = np.zeros((F, KP), BF)
        kt[:, :klens[b]] = k[b, kept[b]].T.astype(BF)
        vt[:, :klens[b]] = v[b, kept[b]].T.astype(BF)
        kTs.append(kt)
        vTs.append(vt)
        val = np.zeros(KP, np.float32)
        val[:klens[b]] = 1.0
        valids.append(np.ascontiguousarray(val.reshape(KT, 128).T)[:, :, None])

    in_maps = []
    for c in range(8):
        b, hg, qg = c // 4, (c // 2) % 2, c % 2
        mc = slice(384 * hg, 384 * (hg + 1))
        qr = slice(QL * qg, QL * (qg + 1))
        in_maps.append({
            "qT": np.ascontiguousarray(q[b, qr].T.astype(BF)),
            "kT": kTs[b],
            "vT": vTs[b],
            "wq": np.ascontiguousarray(w_q[:, mc].astype(BF)),
            "wk": np.ascontiguousarray(w_k[:, mc].astype(BF)),
            "wv": np.ascontiguousarray(w_v[:, mc].astype(BF)),
            "wo": np.ascontiguousarray(w_o[mc, :].astype(BF)),
            "valid": valids[b],
        })

    trace = bool(os.environ.get("KERNEL_TRACE")) or bool(os.environ.get("BASS_TRACE"))
    if trace:
        _ensure_ntff_hook()
    tcores = os.environ.get("KERNEL_TRACE_CORES", "0,1,2,3,4,5,6,7")
    tcores = [int(x) for x in tcores.split(",")]
    try:
        res = bass_utils.run_bass_kernel_spmd(
            nc, in_maps, core_ids=list(range(8)),
            trace=trace,
            trace_cores=tcores if trace else None,
        )
    except Exception:
        if not trace:
            raise
        os.environ["BASS_NEVER_TRACE"] = "1"
        try:
            res = bass_utils.run_bass_kernel_spmd(
                nc, in_maps, core_ids=list(range(8)),
            )
        finally:
            del os.environ["BASS_NEVER_TRACE"]
    last_results = res

    out = np.empty((BZ, SL, F), np.float32)
    for b in range(BZ):
        for qg in range(2):
            c0 = b * 4 + qg          # hg = 0
            c1 = b * 4 + 2 + qg      # hg = 1
            out[b, QL * qg:QL * (qg + 1)] = (
                res.results[c0]["out"] + res.results[c1]["out"] + b_o
            )
    return out


# revision 41
# speedup vs baseline: 1.0168x; 1.0047x over previous
"""Multi-head attention (bz=2, slen=4096, d=768, 12 heads) on 8 trn2 NeuronCores.

Sharding: 8 cores = 2 (batch) x 2 (head halves of 6) x 2 (q halves of 2048).
Each core computes its q-slice of the partial output for its 6 heads; host sums
the two head-half partials per (batch, q-half) and adds b_o.

Key device-side structure (per core):
  - all matmul operands are bf16 (full-rate on PE, half the DMA/SBUF of fp32;
    measured end-to-end rel err ~5e-3 vs the 2e-2 gate); PSUM accumulates f32.
  - projections contract input features (768) on partitions; q/k inputs are fed
    pre-transposed [768, L] so qh^T/kh^T come out feature-major (QK operands),
    while vh is produced position-major (PV stationary operand) by using the
    transposed v as the stationary operand instead.
  - mask handling is exact and free: masked k-positions are gathered away on the
    host (softmax with -1e9 gives exactly 0 in fp32), and right-padding to a
    multiple of 128 is neutralized by a per-position "valid" column appended to
    v, which simultaneously produces the softmax row-sums during PV.
  - scores stay in S^T layout [k-pos partitions, q free]: softmax needs only one
    ACT pass (exp with fused 1/sqrt(64) scale); row-sums fall out of PV; the
    per-(head,q) normalization is applied to attn^T (64 rows) via
    reciprocal_approx_fast + gpsimd partition_broadcast + one DVE multiply.
  - QK packs head pairs into the PE array rows (K=64 x 2 via tile_position).
  - emission braiding: three lanes (projection chunks / attention rounds /
    o-projection) interleave fine-grained steps so the in-order PE and ACT
    queues never head-of-line block each other; o-projection no longer runs
    inside the attention lane (that starved the ACT engine between units).
"""

import os

import numpy as np

_CACHE = {}


def _build(KP):
    import concourse.mybir as mybir
    import concourse.tile as tile
    from concourse import bacc

    F32 = mybir.dt.float32
    BF16 = mybir.dt.bfloat16
    EXP = mybir.ActivationFunctionType.Exp

    F = 768          # model dim
    M = 384          # output features per core (6 heads x 64)
    QL = 2048        # q rows per core
    D = 64           # head dim
    H = 6            # heads per core
    KT = KP // 128   # k tiles
    QCW = 512        # chunk width (q chunks and projection column chunks)
    NQC = QL // QCW

    nc = bacc.Bacc("TRN2", target_bir_lowering=False, debug=False, num_devices=8)

    qT_d = nc.dram_tensor("qT", [F, QL], BF16, kind="ExternalInput").ap()
    kT_d = nc.dram_tensor("kT", [F, KP], BF16, kind="ExternalInput").ap()
    vT_d = nc.dram_tensor("vT", [F, KP], BF16, kind="ExternalInput").ap()
    wq_d = nc.dram_tensor("wq", [F, M], BF16, kind="ExternalInput").ap()
    wk_d = nc.dram_tensor("wk", [F, M], BF16, kind="ExternalInput").ap()
    wv_d = nc.dram_tensor("wv", [F, M], BF16, kind="ExternalInput").ap()
    wo_d = nc.dram_tensor("wo", [M, F], BF16, kind="ExternalInput").ap()
    valid_d = nc.dram_tensor("valid", [128, KT, 1], F32, kind="ExternalInput").ap()
    out_d = nc.dram_tensor("out", [QL, F], F32, kind="ExternalOutput").ap()

    with tile.TileContext(nc) as tc:
        with (
            tc.tile_pool(name="weights", bufs=1) as wp,
            tc.tile_pool(name="acts", bufs=1) as ap_,
            tc.tile_pool(name="ptp", bufs=20) as ptp,
            tc.tile_pool(name="small", bufs=4) as sp,
            tc.tile_pool(name="atp", bufs=2) as atp,
            tc.tile_pool(name="obp", bufs=2) as obp,
            tc.tile_pool(name="xc", bufs=3) as xcp,
            tc.tile_pool(name="xck", bufs=7) as xckp,
            tc.tile_pool(name="sps", bufs=2, space="PSUM") as sps,
            tc.tile_pool(name="acc", bufs=2, space="PSUM") as accp,
            tc.tile_pool(name="prj", bufs=2, space="PSUM") as prjp,
        ):
            # ---- weights: valid up front (tiny); wq/wk/wv/wo ride chunk
            # prefetches so the startup DMA bandwidth goes to the q0/k0/v0
            # chunks the first attention unit gates on
            valid_s = wp.tile([128, KT, 1], F32)
            nc.sync.dma_start(out=valid_s, in_=valid_d)
            wv_s = wp.tile([128, 6, M], BF16)
            wk_s = wp.tile([128, 6, M], BF16)
            wq_s = wp.tile([128, 6, M], BF16)
            wo_s = wp.tile([128, 3, F], BF16)

            def dma_w(dst, src):
                def go():
                    nc.sync.dma_start(
                        out=dst, in_=src.rearrange("(t p) m -> p t m", p=128))
                return go

            def dma_w_split(dst, src):
                # per-output-block pieces: the first projection matmul (m=0)
                # only waits for its own slice of the weights
                def go():
                    for m in range(3):
                        nc.sync.dma_start(
                            out=dst[:, :, m * 128:(m + 1) * 128],
                            in_=src[:, m * 128:(m + 1) * 128]
                            .rearrange("(t p) m -> p t m", p=128))
                return go

            dma_wq = dma_w_split(wq_s, wq_d)
            dma_wk = dma_w_split(wk_s, wk_d)
            dma_wv = dma_w(wv_s, wv_d)
            dma_wo = dma_w(wo_s, wo_d)

            # ---- resident projected activations ----
            qhT = ap_.tile([128, 3, QL], BF16)       # feature-major, head pair p
            khT = ap_.tile([128, 3, KP], BF16)
            vh = ap_.tile([128, KT, H, D + 1], BF16)  # position-major + valid col
            for h in range(H):
                nc.vector.tensor_copy(out=vh[:, :, h, D:D + 1], in_=valid_s)

            # readiness state: attention rounds spin-gate on these so their
            # emission order respects the progressive kh/vh/qh production
            # (kkt is per head-pair m so pair 0 can start before pairs 1/2
            # of the same k chunk have projected)
            st = {"vkt": 0, "kkt": [0, 0, 0], "qdone": set(), "fin": set()}

            class VUnit:
                kind = "v"

                def __init__(self, c0, cw, wdma=None):
                    self.c0 = c0
                    self.cw = cw
                    self.wdma = wdma

                def prefetch(self):
                    for w in self.wdma or ():
                        w()
                    self.xc = xcp.tile([128, 6, QCW], BF16, tag="xc",
                                       name=f"xcv{self.c0}")
                    nc.sync.dma_start(
                        out=self.xc[:, :, :self.cw],
                        in_=vT_d[:, self.c0:self.c0 + self.cw]
                        .rearrange("(t p) n -> p t n", p=128),
                    )

                def gen(self):
                    for t in range(self.cw // 128):
                        kt = self.c0 // 128 + t
                        ps = prjp.tile([128, M], F32, tag="prj")
                        for K in range(6):
                            nc.tensor.matmul(
                                ps,
                                lhsT=self.xc[:, K, t * 128:(t + 1) * 128],
                                rhs=wv_s[:, K, :],
                                start=(K == 0), stop=(K == 5),
                            )
                            if K % 2 == 1 and K < 5:
                                yield
                        nc.vector.tensor_copy(
                            out=vh[:, kt, :, 0:D],
                            in_=ps.rearrange("p (h d) -> p h d", h=H),
                        )
                        st["vkt"] += 1
                        yield

            class XUnit:
                def __init__(self, kind, xd, w_s, dst, c0, cw, wdma=None):
                    self.kind = kind
                    self.xd, self.w_s, self.dst, self.c0 = xd, w_s, dst, c0
                    self.cw = cw
                    self.wdma = wdma

                def prefetch(self):
                    for w in self.wdma or ():
                        w()
                    self.xc = xcp.tile([128, 6, QCW], BF16, tag="xc",
                                       name=f"xcx{id(self)}")
                    if self.kind == "q" and self.c0 == 0:
                        # piecewise so the very first projection matmul can
                        # start after ~0.5MB of DMA instead of the full chunk
                        for k2 in range(3):
                            nc.sync.dma_start(
                                out=self.xc[:, 2 * k2:2 * k2 + 2, :self.cw],
                                in_=self.xd[256 * k2:256 * (k2 + 1),
                                            self.c0:self.c0 + self.cw]
                                .rearrange("(t p) n -> p t n", p=128),
                            )
                    else:
                        nc.sync.dma_start(
                            out=self.xc[:, :, :self.cw],
                            in_=self.xd[:, self.c0:self.c0 + self.cw]
                            .rearrange("(t p) n -> p t n", p=128),
                        )

                def gen(self):
                    for m in range(3):
                        ps = prjp.tile([128, QCW], F32, tag="prj")
                        for K in range(6):
                            nc.tensor.matmul(
                                ps[:, :self.cw],
                                lhsT=self.w_s[:, K, m * 128:(m + 1) * 128],
                                rhs=self.xc[:, K, :self.cw],
                                start=(K == 0), stop=(K == 5),
                            )
                            if K % 2 == 1 and K < 5:
                                yield
                        nc.vector.tensor_copy(
                            out=self.dst[:, m, self.c0:self.c0 + self.cw],
                            in_=ps[:, :self.cw])
                        if self.kind == "q":
                            st["qdone"].add((self.c0 // QCW, m))
                        elif self.kind == "k":
                            st["kkt"][m] += self.cw // 128
                        yield

            # k-projection is split per (chunk, head-pair m): all m=0 pieces
            # emit first so attention unit (qc=0, p=0) unblocks after a short
            # k wave instead of waiting for the full k/v projection stream.
            # The chunk's activations are DMA'd once and stay resident for
            # the later m waves.
            xcks = {}

            class KMUnit:
                kind = "km"

                def __init__(self, ci, c0, cw, m, wdma=None):
                    self.ci, self.c0, self.cw, self.m = ci, c0, cw, m
                    self.wdma = wdma

                def prefetch(self):
                    for w in self.wdma or ():
                        w()
                    if self.m == 0:
                        xcks[self.ci] = xckp.tile(
                            [128, 6, QCW], BF16, tag="xck",
                            name=f"xck{self.ci}")
                        nc.sync.dma_start(
                            out=xcks[self.ci][:, :, :self.cw],
                            in_=kT_d[:, self.c0:self.c0 + self.cw]
                            .rearrange("(t p) n -> p t n", p=128),
                        )

                def gen(self):
                    m, xc = self.m, xcks[self.ci]
                    ps = prjp.tile([128, QCW], F32, tag="prj")
                    for K in range(6):
                        nc.tensor.matmul(
                            ps[:, :self.cw],
                            lhsT=wk_s[:, K, m * 128:(m + 1) * 128],
                            rhs=xc[:, K, :self.cw],
                            start=(K == 0), stop=(K == 5),
                        )
                        if K % 2 == 1 and K < 5:
                            yield
                    nc.vector.tensor_copy(
                        out=khT[:, m, self.c0:self.c0 + self.cw],
                        in_=ps[:, :self.cw])
                    st["kkt"][m] += self.cw // 128
                    yield

            at_tiles = {}

            # ---- streamed attention pipeline ----
            # Two coupled lanes over the same (qc, p, kt) stream: the QK lane
            # emits QK matmuls + exp as soon as kh tiles and a pt buffer are
            # available (keeping the ACT engine continuously fed, even while
            # the PE crunches v/k projections), and the PV lane trails behind
            # gated on vh production, draining pt buffers into the per-unit
            # PSUM accumulators. The pt pool depth bounds the lead. PV needs
            # no PSUM until it runs, so QK of later units overlaps PV of
            # earlier ones without extra PSUM banks.
            units = [(qc, p) for qc in range(NQC) for p in range(3)]
            ptq = []
            stc = {"qk": 0, "pv": 0}
            PT_B = 20
            TOT = len(units) * KT

            def qk_stream():
                for qc, p in units:
                    q0 = qc * QCW
                    while (qc, p) not in st["qdone"]:
                        yield
                    for kt in range(KT):
                        while kt >= st["kkt"][p]:
                            yield
                        lag = min(PT_B - 1, max(1, TOT - stc["qk"]))
                        while stc["qk"] - stc["pv"] >= lag:
                            yield
                        ps = sps.tile([128, 2, QCW], F32, tag="ps")
                        for j in range(2):
                            nc.tensor.matmul(
                                ps[:, j, :],
                                lhsT=khT[j * 64:(j + 1) * 64, p,
                                         kt * 128:(kt + 1) * 128],
                                rhs=qhT[j * 64:(j + 1) * 64, p, q0:q0 + QCW],
                                start=True, stop=True,
                            )
                        pt = ptp.tile([128, 2, QCW], BF16, tag="pt")
                        nc.scalar.activation(pt, ps, EXP, scale=0.125)
                        ptq.append(pt)
                        stc["qk"] += 1
                        yield

            def finalize(qc, p, pacc):
                if qc not in at_tiles:
                    at_tiles[qc] = atp.tile([128, 3, QCW], BF16, tag="at",
                                            name=f"at{qc}")
                at = at_tiles[qc]
                # copy the accumulators out of PSUM right away (frees the
                # pacc bank for the next unit ~1us after the last PV instead
                # of after the whole normalize chain), then normalize from
                # SBUF (the custom-DVE reciprocal misreads PSUM on hw anyway)
                cps, sms = [], []
                for j in range(2):
                    # partition-0 staging for the sums first (the custom-DVE
                    # reciprocal misreads inputs at a nonzero base partition);
                    # emitting these before the big copies starts the recip
                    # chain earlier
                    sm = sp.tile([1, QCW], F32, tag="sm",
                                 name=f"sm{qc}_{p}_{j}")
                    nc.vector.tensor_copy(out=sm, in_=pacc[j][D:D + 1, :])
                    sms.append(sm)
                for j in range(2):
                    cp = sp.tile([D + 1, QCW], F32, tag="cp",
                                 name=f"cp{qc}_{p}_{j}")
                    nc.vector.tensor_copy(out=cp, in_=pacc[j])
                    cps.append(cp)
                for j in range(2):
                    rr = sp.tile([1, QCW], F32, tag="rr")
                    if os.environ.get("KERNEL_EXACT_RECIP"):
                        nc.vector.reciprocal(rr, sms[j])
                    else:
                        nc.vector.reciprocal_approx_fast(out=rr, in_=sms[j])
                    bc = sp.tile([64, QCW], F32, tag="bc")
                    nc.gpsimd.partition_broadcast(bc, rr)
                    nc.vector.tensor_mul(
                        at[j * 64:(j + 1) * 64, p, :],
                        cps[j][0:D, :], bc)
                st["fin"].add((qc, p))
                remaining[qc] -= 1
                if remaining[qc] == 0:
                    ready_oproj.append(OprojUnit(qc))

            def pv_stream():
                for qc, p in units:
                    pacc = None
                    for kt in range(KT):
                        while stc["qk"] <= stc["pv"]:
                            yield
                        while kt >= st["vkt"]:
                            yield
                        if pacc is None:
                            pacc = (accp.tile([D + 1, QCW], F32, tag="acc",
                                              name=f"pa{qc}_{p}"),
                                    accp.tile([D + 1, QCW], F32, tag="acc",
                                              name=f"pb{qc}_{p}"))
                        pt = ptq.pop(0)
                        for j in range(2):
                            nc.tensor.matmul(
                                pacc[j],
                                lhsT=vh[:, kt, 2 * p + j, :],
                                rhs=pt[:, j, :],
                                start=(kt == 0), stop=(kt == KT - 1),
                            )
                        stc["pv"] += 1
                        if kt == KT - 1:
                            finalize(qc, p, pacc)
                            # give the QK lane a head start before the next
                            # unit's first PV (which waits on the pacc copy)
                            yield
                            yield
                        yield

            class OprojUnit:
                kind = "oproj"

                def __init__(self, qc):
                    self.qc = qc

                def gen(self):
                    qc, q0 = self.qc, self.qc * QCW
                    at = at_tiles[qc]
                    for m2 in range(QCW // 128):
                        ob = obp.tile([128, F], F32, tag="ob")
                        for nch in range(2):
                            po = prjp.tile([128, 384], F32, tag="prj")
                            for t in range(3):
                                while (qc, t) not in st["fin"]:
                                    yield
                                nc.tensor.matmul(
                                    po,
                                    lhsT=at[:, t, m2 * 128:(m2 + 1) * 128],
                                    rhs=wo_s[:, t, nch * 384:(nch + 1) * 384],
                                    start=(t == 0), stop=(t == 2),
                                )
                            nc.vector.tensor_copy(
                                out=ob[:, nch * 384:(nch + 1) * 384], in_=po)
                            nc.sync.dma_start(
                                out=out_d[q0 + m2 * 128:q0 + (m2 + 1) * 128,
                                          nch * 384:(nch + 1) * 384],
                                in_=ob[:, nch * 384:(nch + 1) * 384],
                            )
                            yield

            # ---- queues ----
            # head chunks are narrow so the first attention rounds (which
            # gate on k/v tile production) can start as early as possible
            def chunk_spans(L):
                spans, c = [], 0
                for w in (128, 128, 256):
                    if c < L:
                        w2 = min(w, L - c)
                        spans.append((c, w2))
                        c += w2
                while c < L:
                    w2 = min(QCW, L - c)
                    spans.append((c, w2))
                    c += w2
                return spans

            spans = chunk_spans(KP)
            vq = [VUnit(c0, cw, wdma=[dma_wv] if i == 0 else None)
                  for i, (c0, cw) in enumerate(spans)]
            kq = {(i, m): KMUnit(i, c0, cw, m,
                                 wdma=[dma_wk] if (i, m) == (0, 0) else None)
                  for i, (c0, cw) in enumerate(spans) for m in range(3)}
            kq[(min(3, len(spans) - 1), 0)].wdma = \
                (kq[(min(3, len(spans) - 1), 0)].wdma or []) + [dma_wo]
            qunits = [XUnit("q", qT_d, wq_s, qhT, c * QCW, QCW,
                            wdma=[dma_wq] if c == 0 else None)
                      for c in range(NQC)]
            # q0 first (gates attention start), then the m=0 k wave braided
            # with the v chunks, then the m=1 / m=2 k waves and later q
            # chunks (consumed during earlier attention units)
            chunk_q = [qunits[0]]
            for i in range(max(len(vq), len(spans))):
                if i < len(spans):
                    chunk_q.append(kq[(i, 0)])
                if i < len(vq):
                    chunk_q.append(vq[i])
            chunk_q.extend(kq[(i, 1)] for i in range(len(spans)))
            for n, qu in enumerate(qunits[1:], 1):
                chunk_q.append(qu)
                if n == 1:
                    chunk_q.extend(kq[(i, 2)] for i in range(len(spans)))

            def oproj_lane():
                done = 0
                while done < NQC:
                    if ready_oproj:
                        u = ready_oproj.pop(0)
                        # let the finalize DVE chain drain before queueing
                        # oproj matmuls on the PE (head-of-line blocking);
                        # no delay for the last unit (pure tail)
                        if done < NQC - 1:
                            for _ in range(3):
                                yield
                        yield from u.gen()
                        done += 1
                    else:
                        yield

            remaining = {qc: 3 for qc in range(NQC)}
            ready_oproj = []
            idx = {"c": 0, "cpf": 0}

            def ensure_pf(upto):
                while idx["cpf"] < min(upto, len(chunk_q)):
                    u = chunk_q[idx["cpf"]]
                    idx["cpf"] += 1
                    if hasattr(u, "prefetch"):
                        u.prefetch()

            def chunk_lane():
                while idx["c"] < len(chunk_q):
                    u = chunk_q[idx["c"]]
                    idx["c"] += 1
                    ensure_pf(idx["c"] + 2)
                    yield from u.gen()

            ensure_pf(2)
            # braid order per round: QK/exp first (feeds ACT), then PV,
            # then projection chunks, then o-projection
            active = [qk_stream(), pv_stream(), chunk_lane(), oproj_lane()]
            while active:
                nxt = []
                for g in active:
                    try:
                        next(g)
                        nxt.append(g)
                    except StopIteration:
                        pass
                active = nxt

    nc.compile()
    return nc


last_results = None


def _ensure_ntff_hook():
    """Install the axon NTFF profile hook if the image's antenv lacks it.

    trn_agent_boot intends to register this hook (see trn_boot.py step 6); on
    images whose antenv has no axon_hooks module it degrades. Recreate the
    module so trace=True works; silently no-op if anything is unavailable.
    """
    import sys
    import types
    try:
        import antenv.axon_hooks  # noqa: F401
        return
    except ImportError:
        pass
    try:
        import antenv
        from trn_agent_boot.trn_boot import _ntff_profile_via_ctypes
        hook = _ntff_profile_via_ctypes("/opt/axon/libaxon_pjrt.so")
        mod = types.ModuleType("antenv.axon_hooks")
        mod.get_axon_ntff_profile_hook = lambda: hook
        mod.set_axon_ntff_profile_hook = lambda h: None
        sys.modules["antenv.axon_hooks"] = mod
        antenv.axon_hooks = mod
    except Exception:
        pass


def kernel(v, k, q, mask, w_v, w_k, w_q, w_o, b_o):
    global last_results
    import ml_dtypes
    from concourse import bass_utils

    BF = ml_dtypes.bfloat16

    v = np.asarray(v, dtype=np.float32)
    k = np.asarray(k, dtype=np.float32)
    q = np.asarray(q, dtype=np.float32)
    mask = np.asarray(mask)
    w_v = np.asarray(w_v, dtype=np.float32)
    w_k = np.asarray(w_k, dtype=np.float32)
    w_q = np.asarray(w_q, dtype=np.float32)
    w_o = np.asarray(w_o, dtype=np.float32)
    b_o = np.asarray(b_o, dtype=np.float32)

    BZ, SL, F = q.shape
    QL = SL // 2

    kept = [np.flatnonzero(mask[b, 0, 0] == 0) for b in range(BZ)]
    klens = [len(x) for x in kept]
    KP = max(128, -(-max(klens) // 128) * 128)
    KT = KP // 128

    if KP not in _CACHE:
        _CACHE[KP] = _build(KP)
    nc = _CACHE[KP]

    # per-batch gathered/padded transposed k/v and validity tables
    kTs, vTs, valids = [], [], []
    for b in range(BZ):
        kt = np.zeros((F, KP), BF)
        vt = np.zeros((F, KP), BF)
        kt[:, :klens[b]] = k[b, kept[b]].T.astype(BF)
        vt[:, :klens[b]] = v[b, kept[b]].T.astype(BF)
        kTs.append(kt)
        vTs.append(vt)
        val = np.zeros(KP, np.float32)
        val[:klens[b]] = 1.0
        valids.append(np.ascontiguousarray(val.reshape(KT, 128).T)[:, :, None])

    in_maps = []
    for c in range(8):
        b, hg, qg = c // 4, (c // 2) % 2, c % 2
        mc = slice(384 * hg, 384 * (hg + 1))
        qr = slice(QL * qg, QL * (qg + 1))
        in_maps.append({
            "qT": np.ascontiguousarray(q[b, qr].T.astype(BF)),
            "kT": kTs[b],
            "vT": vTs[b],
            "wq": np.ascontiguousarray(w_q[:, mc].astype(BF)),
            "wk": np.ascontiguousarray(w_k[:, mc].astype(BF)),
            "wv": np.ascontiguousarray(w_v[:, mc].astype(BF)),
            "wo": np.ascontiguousarray(w_o[mc, :].astype(BF)),
            "valid": valids[b],
        })

    trace = bool(os.environ.get("KERNEL_TRACE")) or bool(os.environ.get("BASS_TRACE"))
    if trace:
        _ensure_ntff_hook()
    tcores = os.environ.get("KERNEL_TRACE_CORES", "0,1,2,3,4,5,6,7")
    tcores = [int(x) for x in tcores.split(",")]
    try:
        res = bass_utils.run_bass_kernel_spmd(
            nc, in_maps, core_ids=list(range(8)),
            trace=trace,
            trace_cores=tcores if trace else None,
        )
    except Exception:
        if not trace:
            raise
        os.environ["BASS_NEVER_TRACE"] = "1"
        try:
            res = bass_utils.run_bass_kernel_spmd(
                nc, in_maps, core_ids=list(range(8)),
            )
        finally:
            del os.environ["BASS_NEVER_TRACE"]
    last_results = res

    out = np.empty((BZ, SL, F), np.float32)
    for b in range(BZ):
        for qg in range(2):
            c0 = b * 4 + qg          # hg = 0
            c1 = b * 4 + 2 + qg      # hg = 1
            out[b, QL * qg:QL * (qg + 1)] = (
                res.results[c0]["out"] + res.results[c1]["out"] + b_o
            )
    return out
